# revision 19
# baseline (speedup 1.0000x reference)
"""DDPM scheduler kernel for Trainium2 (Bass/Tile), 8-core data parallel.

Computes out = exp(clog[clip(round(t), 0, 1000)]) for t in [0, 1000],
where clog is the cumulative-log-alpha table of the classical DDPM
beta schedule (beta0=1e-4, beta1T/T=0.02, T=1000).

Instead of a 1001-entry table gather (slow on TRN2), we evaluate a
degree-4 polynomial fit of clog(n) (max |err| 3.2e-8 in log domain,
far below the fp32 table's own ~1.1e-5 noise floor vs the exact curve):

    n  = rint(t)                       (DVE, magic-number round-to-nearest-even)
    u  = n / 1024
    P4(u) = SE * [(u+H1)^2 + O1] * [(u+H2)^2 + O2]
    out = exp(P4)

The two quadratic factors are the real-root pair and complex-root pair
of the quartic fit; each is one ACT Square (free fused scale+bias) plus
one scalar add. Per 2.1M-element core: DVE 3 passes, ACT 3 passes,
GPSIMD 1 pass -- every engine under the ~47us HBM roofline.
"""

import numpy as np

import concourse.bacc as bacc
import concourse.mybir as mybir
from concourse.bass_utils import run_bass_kernel_spmd
from concourse.tile import TileContext

N_CORES = 8
TOTAL = 16777216
PER_CORE = TOTAL // N_CORES  # 2097152
P = 128

# fp32 constants (derived offline from the exact fp64 table; see module docstring)
MAGIC = 12582912.0  # 1.5 * 2^23: (t + MAGIC) - MAGIC == rint(t) for 0 <= t < 2^22
SCALE = float(np.float32(2.0**-10))
H1 = float(np.float32(0.0044141756))
O1 = float(np.float32(-1.9481873e-05))
H2 = float(np.float32(47.5497))
O2 = float(np.float32(11728.624))
SE = float(np.float32(-0.0007465615))


def build_nc(per_core: int = PER_CORE, chunks: list[int] | None = None):
    # Ramped chunk widths (free-dim elems per partition): small first chunk
    # so compute starts early (cuts pipeline fill), small last chunk so the
    # final store is short (cuts the tail); big middle chunks amortize
    # per-instruction fixed costs.
    if chunks is None:
        chunks = [1024, 2048, 2560, 2560, 2560, 2560, 2048, 1024]
    assert sum(chunks) * P == per_core
    pad_f = max(chunks)

    # Bacc (not raw Bass): its finalize() runs generate_event_semaphores(),
    # which splits multi-sem waits into InstEventSemaphore chains -- TRN2
    # allows at most 1 sync-wait per compute instruction.
    nc = bacc.Bacc()
    t_in = nc.dram_tensor("t", [per_core], mybir.dt.float32, kind="ExternalInput")
    y_out = nc.dram_tensor("y", [per_core], mybir.dt.float32, kind="ExternalOutput")
    width = per_core // P

    # Per-chunk views: each chunk is a fully CONTIGUOUS DRAM block (strided
    # per-partition layouts measured ~35% lower HBM bandwidth and starved
    # compute mid-kernel). Element permutation is mirrored on the output.
    def chunk_view(dram, base_elems, cw):
        return dram[base_elems : base_elems + P * cw].rearrange(
            "(p f) -> p f", p=P
        )

    AF = mybir.ActivationFunctionType
    OP = mybir.AluOpType
    f32 = mybir.dt.float32

    with TileContext(nc) as tc:
        with (
            tc.tile_pool(name="const", bufs=1) as const_pool,
            tc.tile_pool(name="io", bufs=3) as io_pool,
            tc.tile_pool(name="wk", bufs=3) as wk_pool,
            tc.tile_pool(name="st", bufs=4) as st_pool,
        ):
            b1 = const_pool.tile([P, 1], f32, tag="b1")
            nc.gpsimd.memset(b1[:], H1)
            b2 = const_pool.tile([P, 1], f32, tag="b2")
            nc.gpsimd.memset(b2[:], H2)
            off = 0
            for ci, cw in enumerate(chunks):
                pad = [P, pad_f]
                tt = io_pool.tile([P, cw], f32, tag="t", padded_shape=pad)
                nc.sync.dma_start(tt[:], chunk_view(t_in, off * P, cw))
                # n = rint(t), exact for round-half-to-even (matches jnp.round)
                nc.vector.tensor_scalar(
                    tt[:], tt[:], MAGIC, MAGIC, OP.add, OP.subtract
                )
                # factor 1: (u + H1)^2 + O1   (u = n/1024 via ACT's fused scale)
                y1 = wk_pool.tile([P, cw], f32, tag="y1", padded_shape=pad)
                nc.scalar.activation(y1[:], tt[:], AF.Square, bias=b1[:], scale=SCALE)
                nc.vector.tensor_scalar(y1[:], y1[:], O1, None, OP.add)
                # factor 2: (u + H2)^2 + O2
                y2 = wk_pool.tile([P, cw], f32, tag="y2", padded_shape=pad)
                nc.scalar.activation(y2[:], tt[:], AF.Square, bias=b2[:], scale=SCALE)
                # NOT gpsimd: its tensor_scalar runs ~17x slower than DVE and
                # its SBUF-port sharing stretches concurrent DVE ops to match
                # (measured 2.3us -> 60us).
                nc.vector.tensor_scalar(y2[:], y2[:], O2, None, OP.add)
                # W = factor1 * factor2, into a fresh tile so y1/y2 slots
                # free at the TT read (shorter lifetimes -> deeper pipeline)
                y3 = io_pool.tile([P, cw], f32, tag="y3", padded_shape=pad)
                nc.vector.tensor_tensor(y3[:], y1[:], y2[:], OP.mult)
                # EXP must NOT run in place (HW-garbage when out==in on ACT);
                # deep store pool so EXP never waits on DMA-out completion
                yo = st_pool.tile([P, cw], f32, tag="yo", padded_shape=pad)
                nc.scalar.activation(yo[:], y3[:], AF.Exp, bias=0.0, scale=SE)
                # Stores ride the ACT HWDGE ring (separate FIFO from input
                # loads on the SP ring). Late stores alternate onto the SP
                # ring -- by then the loads have drained, and two rings halve
                # the per-DMA completion-receipt serialization at the tail.
                late = ci >= len(chunks) - 3
                st_engine = nc.sync if (late and ci % 2 == 1) else nc.scalar
                st_engine.dma_start(chunk_view(y_out, off * P, cw), yo[:])
                off += cw
            assert off == width
    # Bacc.finalize() runs compile() (reg alloc, event-sem legalization);
    # run_bass_via_pjrt serializes nc as-is and needs this done.
    nc.finalize()
    return nc


_nc_cache = None


def kernel(t: np.ndarray) -> np.ndarray:
    global _nc_cache
    assert t.shape == (TOTAL,) and t.dtype == np.float32
    if _nc_cache is None:
        _nc_cache = build_nc()
    nc = _nc_cache
    shards = np.ascontiguousarray(t.reshape(N_CORES, PER_CORE))
    in_maps = [{"t": shards[i]} for i in range(N_CORES)]
    res = run_bass_kernel_spmd(nc, in_maps, core_ids=list(range(N_CORES)))
    return np.concatenate([r["y"] for r in res.results])


# revision 20
# speedup vs baseline: 1.0897x; 1.0897x over previous
"""DDPM scheduler kernel for Trainium2 (Bass/Tile), 8-core data parallel.

Computes out = exp(clog[clip(round(t), 0, 1000)]) for t in [0, 1000],
where clog is the cumulative-log-alpha table of the classical DDPM
beta schedule (beta0=1e-4, beta1T/T=0.02, T=1000).

Instead of a 1001-entry table gather (slow on TRN2), we evaluate a
cubic polynomial fit of clog(n) factored into
    P3(u) = SE2 * [(u+H)^2 + O] * (n + F) + BE,   u = n/1024, n = rint(t)
(max |fit err| 9.7e-6 in log domain, at the fp32 table's own ~1.1e-5
noise floor vs the exact curve; F is the far real root scaled by -1024
and rounded to an exact integer so V = n + F is exact in fp32).

Per-chunk engine schedule (Tile framework handles all semaphores):
    DVE : rint (magic-number round-to-nearest-even), Z = Y + O, W = Z*V
    ACT : Y = Square(n/1024 + H), out = Exp(SE2*W + BE)
    V = n + F runs on DVE (variant A) or ACT-Copy (variant B); chunks
    alternate variants in a ratio that equalizes DVE and ACT busy time,
    leaving HBM bandwidth as the only saturated resource.
"""

import numpy as np

import concourse.bacc as bacc
import concourse.mybir as mybir
from concourse.bass_utils import run_bass_kernel_spmd
from concourse.tile import TileContext

N_CORES = 8
TOTAL = 16777216
PER_CORE = TOTAL // N_CORES  # 2097152
P = 128

# fp32 constants (derived offline from the exact fp64 table; see docstring)
MAGIC = 12582912.0  # 1.5 * 2^23: (t + MAGIC) - MAGIC == rint(t) for 0 <= t < 2^22
SCALE = float(np.float32(2.0**-10))
H = float(np.float32(0.0041867206))
O = float(np.float32(0.06839018))
F = 147578.0  # integer: V = rint(t) + F is exact in fp32 (< 2^24)
SE2 = float(np.float32(-7.076394e-05))
BE = float(np.float32(0.7144051))


def build_nc(per_core: int = PER_CORE, plan: list[tuple[int, str]] | None = None):
    # (width, variant) per chunk. Ramped widths: small first chunk so compute
    # starts early, small last chunk so the final store is short. Variant 'A'
    # puts the V = n + F shift on DVE, 'B' on ACT (Copy); the A:B width ratio
    # (~5120:11264) equalizes DVE and ACT busy time.
    if plan is None:
        plan = [
            (1024, "A"), (2048, "B"), (3072, "B"), (3072, "A"),
            (3072, "B"), (2048, "B"), (1024, "B"), (1024, "A"),
        ]
    chunks = [w for w, _ in plan]
    assert sum(chunks) * P == per_core
    pad_f = max(chunks)

    # Bacc (not raw Bass): its finalize() runs generate_event_semaphores(),
    # which splits multi-sem waits into InstEventSemaphore chains -- TRN2
    # allows at most 1 sync-wait per compute instruction.
    nc = bacc.Bacc()
    t_in = nc.dram_tensor("t", [per_core], mybir.dt.float32, kind="ExternalInput")
    y_out = nc.dram_tensor("y", [per_core], mybir.dt.float32, kind="ExternalOutput")

    # Per-chunk views: each chunk is a fully CONTIGUOUS DRAM block (strided
    # per-partition layouts measured ~35% lower HBM bandwidth). The element
    # permutation is mirrored exactly on the output, so any consistent
    # mapping is correct for this purely elementwise kernel.
    def chunk_view(dram, base_elems, cw):
        return dram[base_elems : base_elems + P * cw].rearrange(
            "(p f) -> p f", p=P
        )

    AF = mybir.ActivationFunctionType
    OP = mybir.AluOpType
    f32 = mybir.dt.float32

    with TileContext(nc) as tc:
        with (
            tc.tile_pool(name="const", bufs=1) as const_pool,
            tc.tile_pool(name="io", bufs=3) as io_pool,
            tc.tile_pool(name="wk", bufs=3) as wk_pool,
            tc.tile_pool(name="vv", bufs=2) as vv_pool,
            tc.tile_pool(name="st", bufs=4) as st_pool,
        ):
            bh = const_pool.tile([P, 1], f32, tag="bh")
            nc.gpsimd.memset(bh[:], H)
            bbe = const_pool.tile([P, 1], f32, tag="bbe")
            nc.gpsimd.memset(bbe[:], BE)
            off = 0
            for ci, (cw, variant) in enumerate(plan):
                pad = [P, pad_f]
                tt = io_pool.tile([P, cw], f32, tag="t", padded_shape=pad)
                nc.sync.dma_start(tt[:], chunk_view(t_in, off * P, cw))
                # n = rint(t), exact round-half-to-even (matches jnp.round)
                nc.vector.tensor_scalar(
                    tt[:], tt[:], MAGIC, MAGIC, OP.add, OP.subtract
                )
                # Y = (u + H)^2, u = n/1024 via ACT's fused scale
                y1 = wk_pool.tile([P, cw], f32, tag="y1", padded_shape=pad)
                nc.scalar.activation(y1[:], tt[:], AF.Square, bias=bh[:], scale=SCALE)
                # Z = Y + O (in place; DVE in-place is safe, ACT in-place is NOT)
                nc.vector.tensor_scalar(y1[:], y1[:], O, None, OP.add)
                # V = n + F  (exact: F integer, result < 2^24)
                if variant == "A":
                    # in place on tt (after the Square read); DVE
                    nc.vector.tensor_scalar(tt[:], tt[:], F, None, OP.add)
                    vv = tt
                else:
                    # ACT Copy(scale*x + bias); float bias allowed for Copy
                    vv = vv_pool.tile([P, cw], f32, tag="v", padded_shape=pad)
                    nc.scalar.activation(vv[:], tt[:], AF.Copy, bias=F, scale=1.0)
                # W = Z * V
                y3 = io_pool.tile([P, cw], f32, tag="y3", padded_shape=pad)
                nc.vector.tensor_tensor(y3[:], y1[:], vv[:], OP.mult)
                # out = exp(SE2*W + BE); NOT in place (ACT in-place = garbage)
                yo = st_pool.tile([P, cw], f32, tag="yo", padded_shape=pad)
                nc.scalar.activation(yo[:], y3[:], AF.Exp, bias=bbe[:], scale=SE2)
                # Stores ride the ACT HWDGE ring (separate FIFO from loads on
                # the SP ring); late stores alternate onto the SP ring once
                # the loads have drained, halving tail receipt serialization.
                late = ci >= len(plan) - 3
                st_engine = nc.sync if (late and ci % 2 == 1) else nc.scalar
                st_engine.dma_start(chunk_view(y_out, off * P, cw), yo[:])
                off += cw
    # Bacc.finalize() runs compile() (reg alloc, event-sem legalization);
    # run_bass_via_pjrt serializes nc as-is and needs this done.
    nc.finalize()
    return nc


_nc_cache = None


def kernel(t: np.ndarray) -> np.ndarray:
    global _nc_cache
    assert t.shape == (TOTAL,) and t.dtype == np.float32
    if _nc_cache is None:
        _nc_cache = build_nc()
    nc = _nc_cache
    shards = np.ascontiguousarray(t.reshape(N_CORES, PER_CORE))
    in_maps = [{"t": shards[i]} for i in range(N_CORES)]
    res = run_bass_kernel_spmd(nc, in_maps, core_ids=list(range(N_CORES)))
    return np.concatenate([r["y"] for r in res.results])
